# revision 20
# baseline (speedup 1.0000x reference)
"""Distributed Trainium2 (8 NeuronCore) multi-head attention kernel.

Problem: y = softmax((x Wq)(x Wk)^T * DIM**-0.5) (x Wv) Wo + bo
  x: [4096, 256], 8 heads of dim 32, scale by full-dim**-0.5 (1/16).

Sharding: head-parallel. Each core owns one head h.

v4: keeps v1's proven pipeline skeleton (per-engine contiguous PSUM
score units, ring depth 2, interleaved AV) and adds:
  - 2-way PE row tiling for the score matmuls: q^T/k^T arrive
    host-replicated across partition groups; each score step issues a
    concurrent pair of K=32 matmuls on row tiles (0,0) and (32,0)
    writing two different PSUM banks of the same contiguous 4-j unit.
    Halves the serialized LDWEIGHTS+MATMUL cost per j-tile.
  - exp split ScalarE 20 : VectorE 12 j-tiles; VectorE Schraudolph
    writes through a bitcast AP straight into the bf16 P^T tile (no
    fixup copy). Epilogue (reciprocal + 1/den scaling) on VectorE.
  - a PE warm-up burst at kernel start so the HAM clock gate reaches
    8/8 before the QKV projection.
  - the half-0 output projection emitted between the final AllToAll
    and its scatter so it runs under the collective.
"""

import numpy as np

P = 128          # partitions
N = 4096         # sequence length
DIM = 256        # model dim
H = 8            # heads == cores
D = DIM // H     # head dim, 32
KC = DIM // P    # 2 contraction chunks
NT = N // P      # 32 j-tiles
NCORES = 8
RPC = N // NCORES   # 512 output rows per core
QW = 256         # q-column width per pass
NPASS = 2 * NCORES  # 16
SCALE = DIM ** -0.5

# Schraudolph bf16 fast-exp: bits(exp(s*SCALE)) ~= s*FE_A + FE_B (int16)
FE_A = 128.0 * SCALE * 1.4426950408889634
FE_B = 16256.0 - 4.6

# Per-pass structs: 8 units of 4 j-tiles; True = VectorE unit.
# D units at 1, 4, 6 so the single-buffered D region has >=2 structs
# between reuses. S gets 20 j-tiles, D 12.
UNIT_SCHED = [False, True, False, False, True, False, True, False]

J_ENG = {}
_s = _d = 0
for _u, _dve in enumerate(UNIT_SCHED):
    for _k in range(4):
        _j = 4 * _u + _k
        if _dve:
            J_ENG[_j] = (True, _d)
            _d += 1
        else:
            J_ENG[_j] = (False, _s)
            _s += 1
NSJ, NDJ = _s, _d   # 20, 12

_BUILT = None


def _build():
    from contextlib import ExitStack

    import concourse.mybir as mybir
    import concourse.tile as tile
    from concourse import bacc
    from concourse.masks import make_identity

    f32 = mybir.dt.float32
    bf16 = mybir.dt.bfloat16
    i16 = mybir.dt.int16
    AF = mybir.ActivationFunctionType
    ALU = mybir.AluOpType

    nc = bacc.Bacc("TRN2", target_bir_lowering=False, debug=False,
                   num_devices=NCORES)
    xT = nc.dram_tensor("xT", [DIM, N], bf16, kind="ExternalInput")
    wqr = nc.dram_tensor("wqr", [DIM, P], f32, kind="ExternalInput")
    wkr = nc.dram_tensor("wkr", [DIM, P], f32, kind="ExternalInput")
    wv = nc.dram_tensor("wv", [DIM, D], f32, kind="ExternalInput")
    bqr = nc.dram_tensor("bqr", [P, 1], f32, kind="ExternalInput")
    bkr = nc.dram_tensor("bkr", [P, 1], f32, kind="ExternalInput")
    bv = nc.dram_tensor("bv", [D, 1], f32, kind="ExternalInput")
    wout = nc.dram_tensor("wout", [DIM, DIM], f32, kind="ExternalInput")
    bout = nc.dram_tensor("bout", [1, DIM], f32, kind="ExternalInput")
    out = nc.dram_tensor("out", [RPC, DIM], f32, kind="ExternalOutput")

    with tile.TileContext(nc) as tc, ExitStack() as ctx:
        singles = ctx.enter_context(tc.tile_pool(name="singles", bufs=1))
        sm_pool = ctx.enter_context(tc.tile_pool(name="sm", bufs=3))
        pt_pool = ctx.enter_context(tc.tile_pool(name="ptp", bufs=2))
        # PSUM (8 banks): S pool 2x2 + D region 2 + work 2
        sp_pool = ctx.enter_context(
            tc.tile_pool(name="spp", bufs=2, space="PSUM"))
        ps_sing = ctx.enter_context(
            tc.tile_pool(name="pss", bufs=1, space="PSUM"))
        work_pool = ctx.enter_context(
            tc.tile_pool(name="workp", bufs=2, space="PSUM"))
        dram = ctx.enter_context(
            tc.tile_pool(name="dram", bufs=1, space="DRAM"))

        ones1 = singles.tile([1, P], bf16)
        nc.vector.memset(ones1[:], 1.0)
        ident = singles.tile([P, P], bf16)
        make_identity(nc, ident[:])

        # D-engine score region (also used as HAM warm-up scratch)
        std = ps_sing.tile([P, 4, QW], f32, tag="std", name="std")

        # HAM warm-up: ~40 tiny matmuls keep the PE busy ~2us so the
        # clock gate is at 8/8 when the real work starts (which is as
        # soon as the first x chunks land, ~10.5us).
        for w in range(40):
            nc.tensor.matmul(std[:, 0, 0:64], lhsT=ident[:, 0:P],
                             rhs=ident[:, 0:64], start=True, stop=True)

        # ---------------- constant / input loads ----------------
        # DMA order is the QKV critical path: k weights, first x pair,
        # then the rest. All input DMAs share one ~420 GB/s aggregate
        # pipe, so ordering (not parallelism) is what matters.
        xbf = singles.tile([P, KC, N], bf16)

        def _xpair(q4):
            sl = slice(q4 * (N // 4), (q4 + 1) * (N // 4))
            for c in range(KC):
                nc.sync.dma_start(out=xbf[:, c, sl],
                                  in_=xT[c * P:(c + 1) * P, sl])

        def _ldw(t, cols):
            w32 = singles.tile([P, KC, cols], f32, name=f"w32{t.name}",
                               tag=f"w32{t.name}")
            for c in range(KC):
                nc.sync.dma_start(out=w32[:, c, :], in_=t[c * P:(c + 1) * P, :])
            wbf = singles.tile([P, KC, cols], bf16, name=f"wbf{t.name}",
                               tag=f"wbf{t.name}")
            nc.vector.tensor_copy(wbf[:], w32[:])
            return wbf

        wkbf = _ldw(wkr, P)
        bk_t = singles.tile([P, 1], f32, name="bkt", tag="bkt")
        nc.sync.dma_start(out=bk_t[:], in_=bkr[:, :])
        _xpair(0)
        wqbf = _ldw(wqr, P)
        bq_t = singles.tile([P, 1], f32, name="bqt", tag="bqt")
        nc.sync.dma_start(out=bq_t[:], in_=bqr[:, :])
        wvbf = _ldw(wv, D)
        bv_t = singles.tile([D, 1], f32, name="bvt", tag="bvt")
        nc.sync.dma_start(out=bv_t[:], in_=bv[:, :])
        _xpair(1)
        wobf = _ldw(wout, DIM)
        bo32 = singles.tile([1, DIM], f32)
        nc.sync.dma_start(out=bo32[:], in_=bout[:, :])
        bobf = singles.tile([1, DIM], bf16)
        nc.vector.tensor_copy(bobf[:], bo32[:])
        _xpair(2)
        _xpair(3)

        # ------- QKV projection (128x128 tile mode) ----------------
        # K and V fully, then Q chunk 0; Q chunks 1-7 are emitted inside
        # pass 0 (which only needs q columns 0-255) to shorten the ramp.
        # Evac of each chunk is split ScalarE/VectorE halves so it paces
        # faster than the matmuls.
        qTr = singles.tile([P, N], bf16)
        kTr = singles.tile([P, N], bf16)
        vT = singles.tile([D, N], bf16)
        FT2 = 512
        HF = FT2 // 2

        def emit_qkv_chunk(g, t):
            wbf, bt, dst, m = [(wqbf, bq_t, qTr, P), (wkbf, bk_t, kTr, P),
                               (wvbf, bv_t, vT, D)][g]
            ps = work_pool.tile([P, FT2], f32, tag="wk", name=f"qk{g}_{t}")
            sl0 = t * FT2
            for c in range(KC):
                nc.tensor.matmul(
                    ps[:m, :], lhsT=wbf[:, c, :],
                    rhs=xbf[:, c, sl0:sl0 + FT2],
                    start=(c == 0), stop=(c == KC - 1))
            nc.scalar.activation(dst[:, sl0:sl0 + HF], ps[:m, 0:HF],
                                 AF.Identity, bias=bt[:, 0:1])
            nc.vector.tensor_scalar_add(dst[:, sl0 + HF:sl0 + FT2],
                                        ps[:m, HF:FT2], bt[:])

        # Pre-loop: just enough for pass 0 to begin. Score unit u of
        # pass 0 needs only K chunk u and q columns 0-255; everything
        # else (K2-7, V, Q1-7, v-prep) pipelines into pass 0.
        emit_qkv_chunk(1, 0)
        emit_qkv_chunk(1, 1)
        emit_qkv_chunk(0, 0)

        # ------- v -> [128 j, 32 d] tiles (+ ones col), 32x128 mode -----
        vsb = singles.tile([P, NT, D + 1], bf16)

        def emit_vprep():
            vt = work_pool.tile([P, NT * D], bf16, tag="wk", name="vt")
            for j in range(NT):
                nc.tensor.transpose(vt[:, j * D:(j + 1) * D],
                                    vT[:, j * P:(j + 1) * P],
                                    ident[:D, :D])
            nc.vector.tensor_copy(vsb[:, :, 0:D],
                                  vt.rearrange("p (j d) -> p j d", j=NT))
            nc.vector.memset(vsb[:, :, D:D + 1], 1.0)

        # ---------------- attention main loop ----------------
        otb = singles.tile([P, NCORES, P], bf16)
        a2a_in = [dram.tile([NCORES, D, QW], bf16, name=f"a2ai{h_}",
                            tag=f"a2ai{h_}") for h_ in range(2)]
        a2a_out = [dram.tile([NCORES, D, QW], bf16, name=f"a2ao{h_}",
                             tag=f"a2ao{h_}") for h_ in range(2)]
        osl = [singles.tile([P, KC, QW], bf16, name=f"osl{h_}",
                            tag=f"osl{h_}") for h_ in range(2)]

        def emit_stage(G):
            """Stage otb group G into its a2a input buffer (gpsimd DGE
            queue, off the sync queue's critical path)."""
            hp = G // 4
            for c in (2 * G - 8 * hp, 2 * G + 1 - 8 * hp):
                pp = hp * NCORES + c
                for x in range(2):
                    k = (pp % 2) * 2 + x
                    nc.gpsimd.dma_start(
                        out=a2a_in[hp][c][:, x * P:(x + 1) * P],
                        in_=otb[k * D:(k + 1) * D, G, :])

        def emit_a2a(hp):
            nc.gpsimd.collective_compute(
                "AllToAll", ALU.bypass,
                replica_groups=[list(range(NCORES))],
                ins=[a2a_in[hp][:].opt()], outs=[a2a_out[hp][:].opt()])

        def emit_scatter(hp):
            a2a_flat = a2a_out[hp].rearrange("c d r -> (c d) r")
            for c in range(KC):
                nc.sync.dma_start(
                    out=osl[hp][:, c, :],
                    in_=a2a_flat[c * P:(c + 1) * P, :])

        def emit_half_proj(hp):
            for m2 in range(QW // P):
                mt = hp * (QW // P) + m2
                fo = work_pool.tile([P, DIM], f32, tag="wk")
                for c in range(KC):
                    nc.tensor.matmul(
                        fo[:], lhsT=osl[hp][:, c, m2 * P:(m2 + 1) * P],
                        rhs=wobf[:, c, :], start=(c == 0), stop=False)
                nc.tensor.matmul(fo[:], lhsT=ones1[:], rhs=bobf[:],
                                 start=False, stop=True)
                fout = sm_pool.tile([P, DIM], f32, tag="fout")
                nc.vector.tensor_copy(fout[:], fo[:])
                nc.sync.dma_start(out=out[mt * P:(mt + 1) * P, :],
                                  in_=fout[:])

        def emit_score_unit(p, u, ptq_s, ptq_d):
            """Unit u (js 4u..4u+3): two 2-way row-tiled MM pairs into a
            contiguous [128, 4, 256] 2-bank tile, then one evac op."""
            dve = UNIT_SCHED[u]
            base = (p % NCORES) * RPC + (p // NCORES) * QW
            if dve:
                st = std
            else:
                st = sp_pool.tile([P, 4, QW], f32, tag="sp",
                                  name=f"sp{p}_{u}")
            j0 = 4 * u
            # pair A: j0 -> k0 (bank A, tile 0), j0+2 -> k2 (bank B, t32)
            # pair B: j0+1 -> k1 (bank A),       j0+3 -> k3 (bank B)
            for pb in range(2):
                for i, koff in ((0, 0), (1, 2)):
                    j = j0 + pb + 2 * i
                    nc.tensor.matmul(
                        st[:, pb + koff, :],
                        lhsT=kTr[32 * i:32 * (i + 1), j * P:(j + 1) * P],
                        rhs=qTr[32 * i:32 * (i + 1), base:base + QW],
                        start=True, stop=True,
                        tile_position=(32 * i, 0))
            if dve:
                sl = J_ENG[j0][1]
                nc.vector.tensor_scalar(
                    ptq_d[:, sl:sl + 4, :].bitcast(i16), st[:],
                    scalar1=float(FE_A), scalar2=float(FE_B),
                    op0=ALU.mult, op1=ALU.add)
            else:
                sl = J_ENG[j0][1]
                nc.scalar.activation(ptq_s[:, sl:sl + 4, :], st[:],
                                     AF.Exp, scale=float(SCALE))

        ptqs = [None, None]
        ptqd = [None, None]
        acc = [None, None]
        obq = [None]

        for p in range(NPASS + 1):
            if p < NPASS:
                ptqs[p % 2] = pt_pool.tile([P, NSJ, QW], bf16, tag="pts",
                                           name=f"ptqs{p}")
                ptqd[p % 2] = pt_pool.tile([P, NDJ, QW], bf16, tag="ptd",
                                           name=f"ptqd{p}")
            for ss in range(4):   # super-slots
                if p < NPASS:
                    emit_score_unit(p, 2 * ss, ptqs[p % 2], ptqd[p % 2])
                    emit_score_unit(p, 2 * ss + 1, ptqs[p % 2], ptqd[p % 2])
                if p == 0:
                    for t in (2 * ss + 2, 2 * ss + 3):
                        if t <= 7:
                            emit_qkv_chunk(1, t)   # K, one ss ahead
                    emit_qkv_chunk(2, 2 * ss)      # V
                    emit_qkv_chunk(2, 2 * ss + 1)
                    for t in (2 * ss + 1, 2 * ss + 2):
                        if t <= 7:
                            emit_qkv_chunk(0, t)   # late Q columns
                    if ss == 3:
                        emit_vprep()
                if p > 0:
                    pts, ptd = ptqs[(p - 1) % 2], ptqd[(p - 1) % 2]
                    qb2 = ss // 2
                    if ss % 2 == 0:
                        acc[qb2] = work_pool.tile([P, D + 1], f32, tag="wk",
                                                  name=f"acc{p}_{qb2}")
                    coff = qb2 * P
                    for j in range(16 * (ss % 2), 16 * (ss % 2) + 16):
                        dve, sl_ = J_ENG[j]
                        src = (ptd if dve else pts)[:, sl_, coff:coff + P]
                        nc.tensor.matmul(acc[qb2][:], lhsT=src,
                                         rhs=vsb[:, j, :],
                                         start=(j == 0), stop=(j == NT - 1))
                    if ss % 2 == 1:
                        pp = p - 1
                        k = (pp % 2) * 2 + qb2
                        G = pp // 2
                        if k == 0:
                            obq[0] = sm_pool.tile([P, 4, D], bf16,
                                                  tag="obq", name=f"obq{G}")
                        r = sm_pool.tile([P, 1], f32, tag="r",
                                         name=f"r{p}_{qb2}")
                        nc.vector.reciprocal(r[:], acc[qb2][:, D:D + 1])
                        nc.vector.tensor_scalar_mul(obq[0][:, k, :],
                                                    acc[qb2][:, 0:D],
                                                    r[:])
                        if k == 3:
                            nc.sync.dma_start_transpose(otb[:, G, :],
                                                        obq[0][:])
            # otb group G completes during iteration 2G+2; stage it one
            # iteration later, safely off the critical path.
            if p >= 3 and p % 2 == 1 and (p - 3) // 2 < 7:
                emit_stage((p - 3) // 2)
            if p - 1 == NCORES - 1:
                emit_a2a(0)
                emit_scatter(0)
        emit_stage(7)
        emit_a2a(1)
        emit_half_proj(0)
        emit_scatter(1)
        emit_half_proj(1)

    nc.compile()
    return nc


def _get_built():
    global _BUILT
    if _BUILT is None:
        _BUILT = _build()
    return _BUILT


def make_in_maps(x, w_qkv, b_qkv, w_out, b_out):
    x = np.asarray(x, dtype=np.float32)
    w_qkv = np.asarray(w_qkv, dtype=np.float32)
    b_qkv = np.asarray(b_qkv, dtype=np.float32)
    w_out = np.asarray(w_out, dtype=np.float32)
    b_out = np.asarray(b_out, dtype=np.float32)

    import ml_dtypes
    xT = np.ascontiguousarray(x.T).astype(ml_dtypes.bfloat16)
    wq3 = w_qkv.reshape(DIM, 3, H, D)       # [in, (q|k|v), head, d]
    bq3 = b_qkv.reshape(3, H, D)
    in_maps = []
    for h in range(NCORES):
        in_maps.append({
            "xT": xT,
            "wqr": np.ascontiguousarray(np.tile(wq3[:, 0, h, :], (1, 4))),
            "wkr": np.ascontiguousarray(np.tile(wq3[:, 1, h, :], (1, 4))),
            "wv": np.ascontiguousarray(wq3[:, 2, h, :]),
            "bqr": np.ascontiguousarray(np.tile(bq3[0, h], 4)[:, None]),
            "bkr": np.ascontiguousarray(np.tile(bq3[1, h], 4)[:, None]),
            "bv": np.ascontiguousarray(bq3[2, h][:, None]),
            "wout": np.ascontiguousarray(w_out),
            "bout": np.ascontiguousarray(b_out.reshape(1, DIM)),
        })
    return in_maps


def kernel(x, w_qkv, b_qkv, w_out, b_out):
    from concourse.bass_utils import run_bass_kernel_spmd

    nc = _get_built()
    in_maps = make_in_maps(x, w_qkv, b_qkv, w_out, b_out)
    res = run_bass_kernel_spmd(nc, in_maps, core_ids=list(range(NCORES)))
    return np.concatenate([res.results[i]["out"] for i in range(NCORES)],
                          axis=0)


# revision 23
# speedup vs baseline: 1.0912x; 1.0912x over previous
"""Distributed Trainium2 (8 NeuronCore) multi-head attention kernel.

Problem: y = softmax((x Wq)(x Wk)^T * DIM**-0.5) (x Wv) Wo + bo
  x: [4096, 256], 8 heads of dim 32, scale by full-dim**-0.5 (1/16).

Sharding: head-parallel. Each core owns one head h.

v4: keeps v1's proven pipeline skeleton (per-engine contiguous PSUM
score units, ring depth 2, interleaved AV) and adds:
  - 2-way PE row tiling for the score matmuls: q^T/k^T arrive
    host-replicated across partition groups; each score step issues a
    concurrent pair of K=32 matmuls on row tiles (0,0) and (32,0)
    writing two different PSUM banks of the same contiguous 4-j unit.
    Halves the serialized LDWEIGHTS+MATMUL cost per j-tile.
  - exp split ScalarE 20 : VectorE 12 j-tiles; VectorE Schraudolph
    writes through a bitcast AP straight into the bf16 P^T tile (no
    fixup copy). Epilogue (reciprocal + 1/den scaling) on VectorE.
  - a PE warm-up burst at kernel start so the HAM clock gate reaches
    8/8 before the QKV projection.
  - the half-0 output projection emitted between the final AllToAll
    and its scatter so it runs under the collective.
"""

import numpy as np

P = 128          # partitions
N = 4096         # sequence length
DIM = 256        # model dim
H = 8            # heads == cores
D = DIM // H     # head dim, 32
KC = DIM // P    # 2 contraction chunks
NT = N // P      # 32 j-tiles
NCORES = 8
RPC = N // NCORES   # 512 output rows per core
QW = 256         # q-column width per pass
NPASS = 2 * NCORES  # 16
SCALE = DIM ** -0.5

# Schraudolph bf16 fast-exp: bits(exp(s*SCALE)) ~= s*FE_A + FE_B (int16)
FE_A = 128.0 * SCALE * 1.4426950408889634
FE_B = 16256.0 - 4.6

# Per-pass structs: 8 units of 4 j-tiles; True = VectorE unit.
# D units at 1, 4, 6 so the single-buffered D region has >=2 structs
# between reuses. S gets 20 j-tiles, D 12.
UNIT_SCHED = [False, True, False, False, True, False, True, False]

J_ENG = {}
_s = _d = 0
for _u, _dve in enumerate(UNIT_SCHED):
    for _k in range(4):
        _j = 4 * _u + _k
        if _dve:
            J_ENG[_j] = (True, _d)
            _d += 1
        else:
            J_ENG[_j] = (False, _s)
            _s += 1
NSJ, NDJ = _s, _d   # 20, 12

_BUILT = None


def _build():
    from contextlib import ExitStack

    import concourse.mybir as mybir
    import concourse.tile as tile
    from concourse import bacc
    from concourse.masks import make_identity

    f32 = mybir.dt.float32
    bf16 = mybir.dt.bfloat16
    i16 = mybir.dt.int16
    AF = mybir.ActivationFunctionType
    ALU = mybir.AluOpType

    nc = bacc.Bacc("TRN2", target_bir_lowering=False, debug=False,
                   num_devices=NCORES)
    xT = nc.dram_tensor("xT", [DIM, N], bf16, kind="ExternalInput")
    wqr = nc.dram_tensor("wqr", [DIM, P], f32, kind="ExternalInput")
    wkr = nc.dram_tensor("wkr", [DIM, P], f32, kind="ExternalInput")
    wv = nc.dram_tensor("wv", [DIM, D], f32, kind="ExternalInput")
    bqr = nc.dram_tensor("bqr", [P, 1], f32, kind="ExternalInput")
    bkr = nc.dram_tensor("bkr", [P, 1], f32, kind="ExternalInput")
    bv = nc.dram_tensor("bv", [D, 1], f32, kind="ExternalInput")
    wout = nc.dram_tensor("wout", [DIM, DIM], f32, kind="ExternalInput")
    bout = nc.dram_tensor("bout", [1, DIM], f32, kind="ExternalInput")
    out = nc.dram_tensor("out", [RPC, DIM], f32, kind="ExternalOutput")

    with tile.TileContext(nc) as tc, ExitStack() as ctx:
        singles = ctx.enter_context(tc.tile_pool(name="singles", bufs=1))
        sm_pool = ctx.enter_context(tc.tile_pool(name="sm", bufs=3))
        pt_pool = ctx.enter_context(tc.tile_pool(name="ptp", bufs=2))
        # PSUM (8 banks): S pool 2x2 + D region 2 + work 2
        sp_pool = ctx.enter_context(
            tc.tile_pool(name="spp", bufs=2, space="PSUM"))
        ps_sing = ctx.enter_context(
            tc.tile_pool(name="pss", bufs=1, space="PSUM"))
        work_pool = ctx.enter_context(
            tc.tile_pool(name="workp", bufs=2, space="PSUM"))
        dram = ctx.enter_context(
            tc.tile_pool(name="dram", bufs=1, space="DRAM"))

        ones1 = singles.tile([1, P], bf16)
        nc.vector.memset(ones1[:], 1.0)
        ident = singles.tile([P, P], bf16)
        make_identity(nc, ident[:])

        # D-engine score region (also used as HAM warm-up scratch)
        std = ps_sing.tile([P, 4, QW], f32, tag="std", name="std")

        # HAM warm-up: ~40 tiny matmuls keep the PE busy ~2us so the
        # clock gate is at 8/8 when the real work starts (which is as
        # soon as the first x chunks land, ~10.5us).
        for w in range(40):
            nc.tensor.matmul(std[:, 0, 0:64], lhsT=ident[:, 0:P],
                             rhs=ident[:, 0:64], start=True, stop=True)

        # ---------------- constant / input loads ----------------
        # DMA order is the QKV critical path: k weights, first x pair,
        # then the rest. All input DMAs share one ~420 GB/s aggregate
        # pipe, so ordering (not parallelism) is what matters.
        xbf = singles.tile([P, KC, N], bf16)

        def _xpair(q4):
            sl = slice(q4 * (N // 4), (q4 + 1) * (N // 4))
            for c in range(KC):
                nc.sync.dma_start(out=xbf[:, c, sl],
                                  in_=xT[c * P:(c + 1) * P, sl])

        def _ldw(t, cols):
            w32 = singles.tile([P, KC, cols], f32, name=f"w32{t.name}",
                               tag=f"w32{t.name}")
            for c in range(KC):
                nc.sync.dma_start(out=w32[:, c, :], in_=t[c * P:(c + 1) * P, :])
            wbf = singles.tile([P, KC, cols], bf16, name=f"wbf{t.name}",
                               tag=f"wbf{t.name}")
            nc.vector.tensor_copy(wbf[:], w32[:])
            return wbf

        wkbf = _ldw(wkr, P)
        bk_t = singles.tile([P, 1], f32, name="bkt", tag="bkt")
        nc.sync.dma_start(out=bk_t[:], in_=bkr[:, :])
        _xpair(0)
        wqbf = _ldw(wqr, P)
        bq_t = singles.tile([P, 1], f32, name="bqt", tag="bqt")
        nc.sync.dma_start(out=bq_t[:], in_=bqr[:, :])
        wvbf = _ldw(wv, D)
        bv_t = singles.tile([D, 1], f32, name="bvt", tag="bvt")
        nc.sync.dma_start(out=bv_t[:], in_=bv[:, :])
        _xpair(1)
        wobf = _ldw(wout, DIM)
        bo32 = singles.tile([1, DIM], f32)
        nc.sync.dma_start(out=bo32[:], in_=bout[:, :])
        bobf = singles.tile([1, DIM], bf16)
        nc.vector.tensor_copy(bobf[:], bo32[:])
        _xpair(2)
        _xpair(3)

        # ------- QKV projection (128x128 tile mode) ----------------
        # K and V fully, then Q chunk 0; Q chunks 1-7 are emitted inside
        # pass 0 (which only needs q columns 0-255) to shorten the ramp.
        # Evac of each chunk is split ScalarE/VectorE halves so it paces
        # faster than the matmuls.
        qTr = singles.tile([P, N], bf16)
        kTr = singles.tile([P, N], bf16)
        vT = singles.tile([D, N], bf16)
        FT2 = 512
        HF = FT2 // 2

        def emit_qkv_chunk(g, t, split=False):
            wbf, bt, dst, m = [(wqbf, bq_t, qTr, P), (wkbf, bk_t, kTr, P),
                               (wvbf, bv_t, vT, D)][g]
            ps = work_pool.tile([P, FT2], f32, tag="wk", name=f"qk{g}_{t}")
            sl0 = t * FT2
            for c in range(KC):
                nc.tensor.matmul(
                    ps[:m, :], lhsT=wbf[:, c, :],
                    rhs=xbf[:, c, sl0:sl0 + FT2],
                    start=(c == 0), stop=(c == KC - 1))
            if split:
                nc.scalar.activation(dst[:, sl0:sl0 + HF], ps[:m, 0:HF],
                                     AF.Identity, bias=bt[:, 0:1])
                nc.vector.tensor_scalar_add(dst[:, sl0 + HF:sl0 + FT2],
                                            ps[:m, HF:FT2], bt[:])
            elif (g * 8 + t) % 2 == 0:
                nc.vector.tensor_scalar_add(dst[:, sl0:sl0 + FT2],
                                            ps[:m, :], bt[:])
            else:
                nc.scalar.activation(dst[:, sl0:sl0 + FT2], ps[:m, :],
                                     AF.Identity, bias=bt[:, 0:1])

        # K and V fully (whole-chunk evacs alternating engines, paced by
        # the matmuls), then Q chunk 0; Q chunks 1-7 go into pass 0.
        for t in range(N // FT2):
            emit_qkv_chunk(1, t)        # K
        emit_qkv_chunk(0, 0)            # Q columns 0-511
        for t in range(N // FT2):
            emit_qkv_chunk(2, t)        # V

        # ------- v -> [128 j, 32 d] tiles (+ ones col), 32x128 mode -----
        vsb = singles.tile([P, NT, D + 1], bf16)
        vt = work_pool.tile([P, NT * D], bf16, tag="wk", name="vt")
        for j in range(NT):
            nc.tensor.transpose(vt[:, j * D:(j + 1) * D],
                                vT[:, j * P:(j + 1) * P],
                                ident[:D, :D])
        nc.vector.tensor_copy(vsb[:, :, 0:D],
                              vt.rearrange("p (j d) -> p j d", j=NT))
        nc.vector.memset(vsb[:, :, D:D + 1], 1.0)

        # ---------------- attention main loop ----------------
        otb = singles.tile([P, NCORES, P], bf16)
        a2a_in = [dram.tile([NCORES, D, QW], bf16, name=f"a2ai{h_}",
                            tag=f"a2ai{h_}") for h_ in range(2)]
        a2a_out = [dram.tile([NCORES, D, QW], bf16, name=f"a2ao{h_}",
                             tag=f"a2ao{h_}") for h_ in range(2)]
        osl = [singles.tile([P, KC, QW], bf16, name=f"osl{h_}",
                            tag=f"osl{h_}") for h_ in range(2)]

        def emit_stage(G):
            """Stage otb group G into its a2a input buffer (gpsimd DGE
            queue, off the sync queue's critical path)."""
            hp = G // 4
            for c in (2 * G - 8 * hp, 2 * G + 1 - 8 * hp):
                pp = hp * NCORES + c
                for x in range(2):
                    k = (pp % 2) * 2 + x
                    nc.gpsimd.dma_start(
                        out=a2a_in[hp][c][:, x * P:(x + 1) * P],
                        in_=otb[k * D:(k + 1) * D, G, :])

        def emit_a2a(hp):
            nc.gpsimd.collective_compute(
                "AllToAll", ALU.bypass,
                replica_groups=[list(range(NCORES))],
                ins=[a2a_in[hp][:].opt()], outs=[a2a_out[hp][:].opt()])

        def emit_scatter(hp):
            a2a_flat = a2a_out[hp].rearrange("c d r -> (c d) r")
            for c in range(KC):
                nc.sync.dma_start(
                    out=osl[hp][:, c, :],
                    in_=a2a_flat[c * P:(c + 1) * P, :])

        def emit_half_proj(hp):
            for m2 in range(QW // P):
                mt = hp * (QW // P) + m2
                fo = work_pool.tile([P, DIM], f32, tag="wk")
                for c in range(KC):
                    nc.tensor.matmul(
                        fo[:], lhsT=osl[hp][:, c, m2 * P:(m2 + 1) * P],
                        rhs=wobf[:, c, :], start=(c == 0), stop=False)
                nc.tensor.matmul(fo[:], lhsT=ones1[:], rhs=bobf[:],
                                 start=False, stop=True)
                fout = sm_pool.tile([P, DIM], f32, tag="fout")
                nc.vector.tensor_copy(fout[:], fo[:])
                nc.sync.dma_start(out=out[mt * P:(mt + 1) * P, :],
                                  in_=fout[:])

        def emit_score_unit(p, u, ptq_s, ptq_d):
            """Unit u (js 4u..4u+3): two 2-way row-tiled MM pairs into a
            contiguous [128, 4, 256] 2-bank tile, then one evac op."""
            dve = UNIT_SCHED[u]
            base = (p % NCORES) * RPC + (p // NCORES) * QW
            if dve:
                st = std
            else:
                st = sp_pool.tile([P, 4, QW], f32, tag="sp",
                                  name=f"sp{p}_{u}")
            j0 = 4 * u
            # pair A: j0 -> k0 (bank A, tile 0), j0+2 -> k2 (bank B, t32)
            # pair B: j0+1 -> k1 (bank A),       j0+3 -> k3 (bank B)
            for pb in range(2):
                for i, koff in ((0, 0), (1, 2)):
                    j = j0 + pb + 2 * i
                    nc.tensor.matmul(
                        st[:, pb + koff, :],
                        lhsT=kTr[32 * i:32 * (i + 1), j * P:(j + 1) * P],
                        rhs=qTr[32 * i:32 * (i + 1), base:base + QW],
                        start=True, stop=True,
                        tile_position=(32 * i, 0))
            if dve:
                sl = J_ENG[j0][1]
                nc.vector.tensor_scalar(
                    ptq_d[:, sl:sl + 4, :].bitcast(i16), st[:],
                    scalar1=float(FE_A), scalar2=float(FE_B),
                    op0=ALU.mult, op1=ALU.add)
            else:
                sl = J_ENG[j0][1]
                nc.scalar.activation(ptq_s[:, sl:sl + 4, :], st[:],
                                     AF.Exp, scale=float(SCALE))

        ptqs = [None, None]
        ptqd = [None, None]
        acc = [None, None]
        obq = [None]

        for p in range(NPASS + 1):
            if p < NPASS:
                ptqs[p % 2] = pt_pool.tile([P, NSJ, QW], bf16, tag="pts",
                                           name=f"ptqs{p}")
                ptqd[p % 2] = pt_pool.tile([P, NDJ, QW], bf16, tag="ptd",
                                           name=f"ptqd{p}")
            for ss in range(4):   # super-slots
                if p < NPASS:
                    emit_score_unit(p, 2 * ss, ptqs[p % 2], ptqd[p % 2])
                    emit_score_unit(p, 2 * ss + 1, ptqs[p % 2], ptqd[p % 2])
                if p == 0:
                    for t in range(2 * ss + 1, min(2 * ss + 3, 8)):
                        emit_qkv_chunk(0, t)   # late Q columns
                if p > 0:
                    pts, ptd = ptqs[(p - 1) % 2], ptqd[(p - 1) % 2]
                    qb2 = ss // 2
                    if ss % 2 == 0:
                        acc[qb2] = work_pool.tile([P, D + 1], f32, tag="wk",
                                                  name=f"acc{p}_{qb2}")
                    coff = qb2 * P
                    for j in range(16 * (ss % 2), 16 * (ss % 2) + 16):
                        dve, sl_ = J_ENG[j]
                        src = (ptd if dve else pts)[:, sl_, coff:coff + P]
                        nc.tensor.matmul(acc[qb2][:], lhsT=src,
                                         rhs=vsb[:, j, :],
                                         start=(j == 0), stop=(j == NT - 1))
                    if ss % 2 == 1:
                        pp = p - 1
                        k = (pp % 2) * 2 + qb2
                        G = pp // 2
                        if k == 0:
                            obq[0] = sm_pool.tile([P, 4, D], bf16,
                                                  tag="obq", name=f"obq{G}")
                        r = sm_pool.tile([P, 1], f32, tag="r",
                                         name=f"r{p}_{qb2}")
                        nc.vector.reciprocal(r[:], acc[qb2][:, D:D + 1])
                        nc.vector.tensor_scalar_mul(obq[0][:, k, :],
                                                    acc[qb2][:, 0:D],
                                                    r[:])
                        if k == 3:
                            nc.sync.dma_start_transpose(otb[:, G, :],
                                                        obq[0][:])
            # otb group G completes during iteration 2G+2; stage it one
            # iteration later, safely off the critical path.
            if p >= 3 and p % 2 == 1 and (p - 3) // 2 < 7:
                emit_stage((p - 3) // 2)
            if p - 1 == NCORES - 1:
                emit_a2a(0)
                emit_scatter(0)
        emit_stage(7)
        emit_a2a(1)
        emit_half_proj(0)
        emit_scatter(1)
        emit_half_proj(1)

    nc.compile()
    return nc


def _get_built():
    global _BUILT
    if _BUILT is None:
        _BUILT = _build()
    return _BUILT


def make_in_maps(x, w_qkv, b_qkv, w_out, b_out):
    x = np.asarray(x, dtype=np.float32)
    w_qkv = np.asarray(w_qkv, dtype=np.float32)
    b_qkv = np.asarray(b_qkv, dtype=np.float32)
    w_out = np.asarray(w_out, dtype=np.float32)
    b_out = np.asarray(b_out, dtype=np.float32)

    import ml_dtypes
    xT = np.ascontiguousarray(x.T).astype(ml_dtypes.bfloat16)
    wq3 = w_qkv.reshape(DIM, 3, H, D)       # [in, (q|k|v), head, d]
    bq3 = b_qkv.reshape(3, H, D)
    in_maps = []
    for h in range(NCORES):
        in_maps.append({
            "xT": xT,
            "wqr": np.ascontiguousarray(np.tile(wq3[:, 0, h, :], (1, 4))),
            "wkr": np.ascontiguousarray(np.tile(wq3[:, 1, h, :], (1, 4))),
            "wv": np.ascontiguousarray(wq3[:, 2, h, :]),
            "bqr": np.ascontiguousarray(np.tile(bq3[0, h], 4)[:, None]),
            "bkr": np.ascontiguousarray(np.tile(bq3[1, h], 4)[:, None]),
            "bv": np.ascontiguousarray(bq3[2, h][:, None]),
            "wout": np.ascontiguousarray(w_out),
            "bout": np.ascontiguousarray(b_out.reshape(1, DIM)),
        })
    return in_maps


def kernel(x, w_qkv, b_qkv, w_out, b_out):
    from concourse.bass_utils import run_bass_kernel_spmd

    nc = _get_built()
    in_maps = make_in_maps(x, w_qkv, b_qkv, w_out, b_out)
    res = run_bass_kernel_spmd(nc, in_maps, core_ids=list(range(NCORES)))
    return np.concatenate([res.results[i]["out"] for i in range(NCORES)],
                          axis=0)
